# revision 4
# baseline (speedup 1.0000x reference)
"""3-level 1D DWT (12-tap analysis filter bank, stride 2, pywt 'zero' mode)
for x:(16,64,16384) f32 on 8 trn2 NeuronCores.

Strategy: data-parallel over the 1024 (B,C) rows -> 128 rows/core. On each
core the signal lives position-major: SBUF chunk layout [p, (blk, r)] where
partition p = position%128, blk = position//128, r = row. The host builds
this layout (transposes are free host-side). Each level's stride-2 conv is
then a banded matmul: an output chunk of 128 positions is
  out[jj, r] = sum_d  W_d[p, jj] * in_chunk(2c+d)[p, r],   d in {-1,0,+1}
with three 128x128 stationary band matrices per filter, accumulated in PSUM.
lo outputs are copied back to SBUF in exactly the chunk layout the next level
reads; hi (and final lo) are staged and DMA'd out position-major, inverted on
the host. Zero blocks around each buffer reproduce pywt zero-padding.
"""

import numpy as np

import concourse.bacc as bacc
import concourse.mybir as mybir
from concourse import bass_utils
from concourse.tile import TileContext

F32 = mybir.dt.float32
F32R = mybir.dt.float32r

N_CORES = 8
R = 128           # rows per core
L = 12            # filter taps
N0 = 16384        # input signal length

# per-level geometry: out chunk c' (128 outputs) reads input chunks
# s = 2c' + delta, delta in {-1,0,1}; groups of 4 out chunks -> N=512 matmuls.
# (valid_out_chunks, full_groups); partial group covers 2 chunks, 1 copied.
LEVELS = [
    dict(valid=65, fg=16),   # N=16384 -> outsize 8197
    dict(valid=33, fg=8),    # 8197 -> 4104
    dict(valid=17, fg=4),    # 4104 -> 2057
]
OUTSIZES = [8197, 4104, 2057]

X_BLOCKS = 134            # 2 zero | 128 signal | 4 zero
LO1_BLOCKS = 70           # 2 zero | chunks 0..64 | 3 zero  (blocks 2..66 written)
LO2_BLOCKS = 38           # 2 zero | chunks 0..32 | 3 zero

_COMPILED = None


def _build():
    nc = bacc.Bacc(
        "TRN2",
        target_bir_lowering=False,
        debug=False,
        enable_asserts=False,
        num_devices=N_CORES,
    )
    x_in = nc.dram_tensor("x_in", [R, X_BLOCKS * 128], F32R, kind="ExternalInput")
    w_in = nc.dram_tensor("w_in", [R, 6 * 128], F32R, kind="ExternalInput")
    outs = {}
    outs["hi1"] = nc.dram_tensor("hi1_out", [R, 65 * 128], F32, kind="ExternalOutput")
    outs["hi2"] = nc.dram_tensor("hi2_out", [R, 33 * 128], F32, kind="ExternalOutput")
    outs["hi3"] = nc.dram_tensor("hi3_out", [R, 17 * 128], F32, kind="ExternalOutput")
    outs["lo3"] = nc.dram_tensor("lo3_out", [R, 17 * 128], F32, kind="ExternalOutput")

    with TileContext(nc) as tc:
        with (
            tc.tile_pool(name="const", bufs=1) as cpool,
            tc.tile_pool(name="xg", bufs=4) as xpool,
            tc.tile_pool(name="lobuf", bufs=1) as lpool,
            tc.tile_pool(name="stage", bufs=1) as spool,
            tc.tile_pool(name="psum", bufs=3, space="PSUM") as ppool,
        ):
            w_sb = cpool.tile([128, 6 * 128], F32R, tag="w")
            nc.sync.dma_start(w_sb[:], w_in[:])

            lo1 = lpool.tile([128, LO1_BLOCKS * 128], F32R, tag="lo1")
            lo2 = lpool.tile([128, LO2_BLOCKS * 128], F32R, tag="lo2")
            hi1s = spool.tile([128, 65 * 128], F32, tag="hi1s")
            hi2s = spool.tile([128, 33 * 128], F32, tag="hi2s")
            hi3s = spool.tile([128, 17 * 128], F32, tag="hi3s")
            lo3s = spool.tile([128, 17 * 128], F32, tag="lo3s")

            # zero pads around the lo buffers (head: chunk -1; tail: beyond
            # the last computed chunk). pywt zero-padding falls out of these.
            # (memset can't target f32r, so memset f32 scratch + cast-copy.)
            zscratch = cpool.tile([128, 3 * 128], F32, tag="zs")
            nc.gpsimd.memset(zscratch[:], 0.0)
            nc.vector.tensor_copy(lo1[:, 0 : 2 * 128], zscratch[:, 0 : 2 * 128])
            nc.vector.tensor_copy(lo1[:, 67 * 128 : 70 * 128], zscratch[:])
            nc.vector.tensor_copy(lo2[:, 0 : 2 * 128], zscratch[:, 0 : 2 * 128])
            nc.vector.tensor_copy(lo2[:, 35 * 128 : 38 * 128], zscratch[:])

            def run_level(lv, in_buf, lo_dst, lo_dst_blk0, hi_dst):
                """in_buf: None -> stream from x_in (level 1), else resident
                SBUF buffer whose block b holds input chunk b-2."""
                cfg = LEVELS[lv]
                groups = [(m, 4) for m in range(cfg["fg"])] + [(cfg["fg"], 2)]
                for m, tcnt in groups:
                    nblk = 2 * tcnt + 2
                    if in_buf is None:
                        src = xpool.tile([128, nblk * 128], F32R, tag="xg")
                        nc.sync.dma_start(
                            src[:], x_in[:, 8 * m * 128 : (8 * m + nblk) * 128]
                        )
                        v = src[:]
                    else:
                        v = in_buf[:, 8 * m * 128 : (8 * m + nblk) * 128]
                    # view [p, t, u, k]: (t,u) selects input chunk 8m-2+2t+u
                    v = v.rearrange("p (t u k) -> p t u k", u=2, k=128)
                    rhs = {
                        -1: v[:, 0:tcnt, 1, :],
                        0: v[:, 1 : tcnt + 1, 0, :],
                        1: v[:, 1 : tcnt + 1, 1, :],
                    }
                    n = tcnt * 128
                    ncopy = min(4, cfg["valid"] - 4 * m)  # chunks worth copying
                    for f in range(2):  # 0 = lo, 1 = hi
                        ps = ppool.tile([128, 512], F32, tag=f"ps{f}")
                        for di, d in enumerate((-1, 0, 1)):
                            wsl = w_sb[:, (3 * f + di) * 128 : (3 * f + di + 1) * 128]
                            nc.tensor.matmul(
                                ps[:, 0:n],
                                wsl,
                                rhs[d],
                                start=(di == 0),
                                stop=(di == 2),
                            )
                        cw = ncopy * 128
                        if f == 0:
                            o = (4 * m + lo_dst_blk0) * 128
                            nc.vector.tensor_copy(lo_dst[:, o : o + cw], ps[:, 0:cw])
                        else:
                            o = 4 * m * 128
                            nc.scalar.copy(hi_dst[:, o : o + cw], ps[:, 0:cw])

            run_level(0, None, lo1, 2, hi1s)
            run_level(1, lo1, lo2, 2, hi2s)
            run_level(2, lo2, lo3s, 0, hi3s)

            nc.scalar.dma_start(outs["hi1"][:], hi1s[:])
            nc.scalar.dma_start(outs["hi2"][:], hi2s[:])
            nc.scalar.dma_start(outs["hi3"][:], hi3s[:])
            nc.scalar.dma_start(outs["lo3"][:], lo3s[:])

    nc.compile()
    return nc


def get_compiled():
    global _COMPILED
    if _COMPILED is None:
        _COMPILED = _build()
    return _COMPILED


def make_weights(hac: np.ndarray) -> np.ndarray:
    """Six 128x128 band matrices [p, (f,delta), jj] flattened to [128, 768]."""
    hac = np.asarray(hac, dtype=np.float32)
    sign = np.where(np.arange(L) % 2 == 0, -1.0, 1.0).astype(np.float32)
    h0 = hac
    h1 = hac[::-1] * sign
    W = np.zeros((128, 6, 128), dtype=np.float32)
    jj = np.arange(128)
    p = np.arange(128)[:, None]
    for f, h in enumerate((h0, h1)):
        for di, off in enumerate((-118, 10, 138)):
            i = p + off - 2 * jj  # tap index
            mask = (i >= 0) & (i < L)
            W[:, 3 * f + di, :][mask] = h[np.clip(i, 0, L - 1)][mask]
    return W.reshape(128, 768)


def make_core_input(x_rows: np.ndarray) -> np.ndarray:
    """x_rows: [128, 16384] -> position-major padded layout [128, X_BLOCKS*128]."""
    A = np.zeros((128, X_BLOCKS, 128), dtype=np.float32)
    # A[p, c+2, r] = x_rows[r, 128c + p]
    A[:, 2:130, :] = x_rows.reshape(128, 128, 128).transpose(2, 1, 0)
    return np.ascontiguousarray(A.reshape(128, X_BLOCKS * 128))


def unpack_out(a: np.ndarray, outsize: int) -> np.ndarray:
    """[128, C*128] position-major -> [128 rows, outsize]."""
    C = a.shape[1] // 128
    return a.reshape(128, C, 128).transpose(2, 1, 0).reshape(128, C * 128)[:, :outsize]


def run_cores(x: np.ndarray, hac: np.ndarray, trace: bool = False):
    x = np.asarray(x, dtype=np.float32)
    B, Ch, N = x.shape
    rows = x.reshape(B * Ch, N)
    W = make_weights(hac)
    in_maps = [
        {
            "x_in": make_core_input(rows[k * R : (k + 1) * R]),
            "w_in": W,
        }
        for k in range(N_CORES)
    ]
    nc = get_compiled()
    res = bass_utils.run_bass_kernel_spmd(
        nc, in_maps, core_ids=list(range(N_CORES)), trace=trace
    )
    names = ["lo3_out", "hi1_out", "hi2_out", "hi3_out"]
    sizes = [2057, 8197, 4104, 2057]
    full = []
    for name, sz in zip(names, sizes):
        parts = [unpack_out(res.results[k][name], sz) for k in range(N_CORES)]
        full.append(np.concatenate(parts, axis=0).reshape(B, Ch, sz))
    return tuple(full), res


def kernel(x: np.ndarray, hac: np.ndarray):
    out, _ = run_cores(x, hac, trace=False)
    return out


# revision 5
# speedup vs baseline: 1.2125x; 1.2125x over previous
"""3-level 1D DWT (12-tap analysis filter bank, stride 2, pywt 'zero' mode)
for x:(16,64,16384) f32 on 8 trn2 NeuronCores.

Strategy: data-parallel over the 1024 (B,C) rows -> 128 rows/core. On each
core the signal lives position-major: SBUF chunk layout [p, (blk, r)] where
partition p = position%128, blk = position//128, r = row. The host builds
this layout (transposes are free host-side). Each level's stride-2 conv is
then a banded matmul: an output chunk of 128 positions is
  out[jj, r] = sum_d  W_d[p, jj] * in_chunk(2c+d)[p, r],   d in {-1,0,+1}
with three 128x128 stationary band matrices per filter, accumulated in PSUM.
lo outputs are copied back to SBUF in exactly the chunk layout the next level
reads; hi (and final lo) are staged and DMA'd out position-major, inverted on
the host. Zero blocks around each buffer reproduce pywt zero-padding.
"""

import numpy as np

import concourse.bacc as bacc
import concourse.mybir as mybir
from concourse import bass_utils
from concourse.tile import TileContext

F32 = mybir.dt.float32
F32R = mybir.dt.float32r
F16 = mybir.dt.float16

N_CORES = 8
R = 128           # rows per core
L = 12            # filter taps
N0 = 16384        # input signal length

# per-level geometry: out chunk c' (128 outputs) reads input chunks
# s = 2c' + delta, delta in {-1,0,1}; groups of 4 out chunks -> N=512 matmuls.
# (valid_out_chunks, full_groups); partial group covers 2 chunks, 1 copied.
LEVELS = [
    dict(valid=65, fg=16),   # N=16384 -> outsize 8197
    dict(valid=33, fg=8),    # 8197 -> 4104
    dict(valid=17, fg=4),    # 4104 -> 2057
]
OUTSIZES = [8197, 4104, 2057]

X_BLOCKS = 134            # 2 zero | 128 signal | 4 zero
LO1_BLOCKS = 70           # 2 zero | chunks 0..64 | 3 zero  (blocks 2..66 written)
LO2_BLOCKS = 38           # 2 zero | chunks 0..32 | 3 zero

_COMPILED = None


def _build():
    nc = bacc.Bacc(
        "TRN2",
        target_bir_lowering=False,
        debug=False,
        enable_asserts=False,
        num_devices=N_CORES,
    )
    x_in = nc.dram_tensor("x_in", [R, X_BLOCKS * 128], F16, kind="ExternalInput")
    w_in = nc.dram_tensor("w_in", [R, 6 * 128], F16, kind="ExternalInput")
    outs = {}
    outs["hi1"] = nc.dram_tensor("hi1_out", [R, 65 * 128], F32, kind="ExternalOutput")
    outs["hi2"] = nc.dram_tensor("hi2_out", [R, 33 * 128], F32, kind="ExternalOutput")
    outs["hi3"] = nc.dram_tensor("hi3_out", [R, 17 * 128], F32, kind="ExternalOutput")
    outs["lo3"] = nc.dram_tensor("lo3_out", [R, 17 * 128], F32, kind="ExternalOutput")

    with TileContext(nc) as tc:
        with (
            tc.tile_pool(name="const", bufs=1) as cpool,
            tc.tile_pool(name="xg", bufs=4) as xpool,
            tc.tile_pool(name="lobuf", bufs=1) as lpool,
            tc.tile_pool(name="stage", bufs=1) as spool,
            tc.tile_pool(name="psum", bufs=3, space="PSUM") as ppool,
        ):
            w_sb = cpool.tile([128, 6 * 128], F16, tag="w")
            nc.sync.dma_start(w_sb[:], w_in[:])

            lo1 = lpool.tile([128, LO1_BLOCKS * 128], F16, tag="lo1")
            lo2 = lpool.tile([128, LO2_BLOCKS * 128], F16, tag="lo2")
            hi1s = spool.tile([128, 65 * 128], F32, tag="hi1s")
            hi2s = spool.tile([128, 33 * 128], F32, tag="hi2s")
            hi3s = spool.tile([128, 17 * 128], F32, tag="hi3s")
            lo3s = spool.tile([128, 17 * 128], F32, tag="lo3s")

            # zero pads around the lo buffers (head: chunk -1; tail: beyond
            # the last computed chunk). pywt zero-padding falls out of these.
            # (memset can't target f32r, so memset f32 scratch + cast-copy.)
            zscratch = cpool.tile([128, 3 * 128], F32, tag="zs")
            nc.gpsimd.memset(zscratch[:], 0.0)
            nc.vector.tensor_copy(lo1[:, 0 : 2 * 128], zscratch[:, 0 : 2 * 128])
            nc.vector.tensor_copy(lo1[:, 67 * 128 : 70 * 128], zscratch[:])
            nc.vector.tensor_copy(lo2[:, 0 : 2 * 128], zscratch[:, 0 : 2 * 128])
            nc.vector.tensor_copy(lo2[:, 35 * 128 : 38 * 128], zscratch[:])

            def run_level(lv, in_buf, lo_dst, lo_dst_blk0, hi_dst):
                """in_buf: None -> stream from x_in (level 1), else resident
                SBUF buffer whose block b holds input chunk b-2."""
                cfg = LEVELS[lv]
                groups = [(m, 4) for m in range(cfg["fg"])] + [(cfg["fg"], 2)]
                for m, tcnt in groups:
                    nblk = 2 * tcnt + 2
                    if in_buf is None:
                        src = xpool.tile([128, nblk * 128], F16, tag="xg")
                        nc.sync.dma_start(
                            src[:], x_in[:, 8 * m * 128 : (8 * m + nblk) * 128]
                        )
                        v = src[:]
                    else:
                        v = in_buf[:, 8 * m * 128 : (8 * m + nblk) * 128]
                    # view [p, t, u, k]: (t,u) selects input chunk 8m-2+2t+u
                    v = v.rearrange("p (t u k) -> p t u k", u=2, k=128)
                    rhs = {
                        -1: v[:, 0:tcnt, 1, :],
                        0: v[:, 1 : tcnt + 1, 0, :],
                        1: v[:, 1 : tcnt + 1, 1, :],
                    }
                    n = tcnt * 128
                    ncopy = min(4, cfg["valid"] - 4 * m)  # chunks worth copying
                    for f in range(2):  # 0 = lo, 1 = hi
                        ps = ppool.tile([128, 512], F32, tag=f"ps{f}")
                        for di, d in enumerate((-1, 0, 1)):
                            wsl = w_sb[:, (3 * f + di) * 128 : (3 * f + di + 1) * 128]
                            nc.tensor.matmul(
                                ps[:, 0:n],
                                wsl,
                                rhs[d],
                                start=(di == 0),
                                stop=(di == 2),
                            )
                        cw = ncopy * 128
                        if f == 0:
                            o = (4 * m + lo_dst_blk0) * 128
                            nc.vector.tensor_copy(lo_dst[:, o : o + cw], ps[:, 0:cw])
                        else:
                            o = 4 * m * 128
                            nc.scalar.copy(hi_dst[:, o : o + cw], ps[:, 0:cw])

            run_level(0, None, lo1, 2, hi1s)
            run_level(1, lo1, lo2, 2, hi2s)
            run_level(2, lo2, lo3s, 0, hi3s)

            nc.scalar.dma_start(outs["hi1"][:], hi1s[:])
            nc.scalar.dma_start(outs["hi2"][:], hi2s[:])
            nc.scalar.dma_start(outs["hi3"][:], hi3s[:])
            nc.scalar.dma_start(outs["lo3"][:], lo3s[:])

    nc.compile()
    return nc


def get_compiled():
    global _COMPILED
    if _COMPILED is None:
        _COMPILED = _build()
    return _COMPILED


def make_weights(hac: np.ndarray) -> np.ndarray:
    """Six 128x128 band matrices [p, (f,delta), jj] flattened to [128, 768]."""
    hac = np.asarray(hac, dtype=np.float32)
    sign = np.where(np.arange(L) % 2 == 0, -1.0, 1.0).astype(np.float32)
    h0 = hac
    h1 = hac[::-1] * sign
    W = np.zeros((128, 6, 128), dtype=np.float32)
    jj = np.arange(128)
    p = np.arange(128)[:, None]
    for f, h in enumerate((h0, h1)):
        for di, off in enumerate((-118, 10, 138)):
            i = p + off - 2 * jj  # tap index
            mask = (i >= 0) & (i < L)
            W[:, 3 * f + di, :][mask] = h[np.clip(i, 0, L - 1)][mask]
    return W.reshape(128, 768).astype(np.float16)


def make_core_input(x_rows: np.ndarray) -> np.ndarray:
    """x_rows: [128, 16384] -> position-major padded fp16 layout."""
    A = np.zeros((128, X_BLOCKS, 128), dtype=np.float16)
    # A[p, c+2, r] = x_rows[r, 128c + p]
    A[:, 2:130, :] = x_rows.reshape(128, 128, 128).transpose(2, 1, 0).astype(np.float16)
    return np.ascontiguousarray(A.reshape(128, X_BLOCKS * 128))


def unpack_out(a: np.ndarray, outsize: int) -> np.ndarray:
    """[128, C*128] position-major -> [128 rows, outsize]."""
    C = a.shape[1] // 128
    return a.reshape(128, C, 128).transpose(2, 1, 0).reshape(128, C * 128)[:, :outsize]


def run_cores(x: np.ndarray, hac: np.ndarray, trace: bool = False):
    x = np.asarray(x, dtype=np.float32)
    B, Ch, N = x.shape
    rows = x.reshape(B * Ch, N)
    W = make_weights(hac)
    in_maps = [
        {
            "x_in": make_core_input(rows[k * R : (k + 1) * R]),
            "w_in": W,
        }
        for k in range(N_CORES)
    ]
    nc = get_compiled()
    res = bass_utils.run_bass_kernel_spmd(
        nc, in_maps, core_ids=list(range(N_CORES)), trace=trace
    )
    names = ["lo3_out", "hi1_out", "hi2_out", "hi3_out"]
    sizes = [2057, 8197, 4104, 2057]
    full = []
    for name, sz in zip(names, sizes):
        parts = [unpack_out(res.results[k][name], sz) for k in range(N_CORES)]
        full.append(np.concatenate(parts, axis=0).reshape(B, Ch, sz))
    return tuple(full), res


def kernel(x: np.ndarray, hac: np.ndarray):
    out, _ = run_cores(x, hac, trace=False)
    return out
